# revision 25
# baseline (speedup 1.0000x reference)
"""Char-level BiLSTM embedder on 8 NeuronCores (Trainium2, Bass/Tile).

Computation: x[B=32,T=128,L=16] char ids -> embed[E=512] -> fwd+bwd LSTM(H=256)
over the L=16 chars of each of the N=B*T=4096 independent words -> final hidden
states concatenated -> y[B,T,2H=512].

Strategy (v2 — balanced 4-engine pipeline):
  - Data parallel over N: 512 words per core.
  - Embedding lookup + input projection + bias fused on HOST into a single
    [V=128, 4H] LUT per direction; on device the per-step input contribution
    is a K=128 bf16 matmul with a one-hot rhs, accumulated in PSUM.
  - Recurrent matmuls run in fp8e4m3 with perf_mode=DoubleRow: one K=256
    matmul per gate chunk (~1.44x over bf16 at FD=512). h is stored fp8
    scaled by 64 and w_hh scaled by 16; the LUT carries the matching 2^10
    scale and activations divide it back out (exact powers of two).
  - Activation work split across engines:
      ACT:  sigmoid(i,f) one batched op + sigmoid(o), reading PSUM with
            scale=2^-10.
      DVE:  custom fused ops (registered in dve_ops at import):
            TANH_PS_ANT  = deg-5 odd tanh poly straight from PSUM
            TANH_MUL_ANT = tanh(c)*sig_o fused (writes recurrent h in fp8)
            plus scalar_tensor_tensor (4x-capable) for the cell update.
    Gate pre-activations are tiny (|x| <= 0.5) so the deg-5 poly is exact
    to ~4e-5.
  - Gate order permuted to (i,f,o,g); psum_a=[i,f] psum_b=[g,o] so tanh(g)
    can drain early while o still accumulates.
  - fwd and bwd directions interleave per step to hide recurrence latency.
"""

import sys

sys.path.insert(0, "/opt/trn_rl_repo")

import numpy as np
import concourse.bass as bass
import concourse.bacc as bacc
import concourse.mybir as mybir
import concourse.tile as tile
from concourse.bass_utils import run_bass_kernel_spmd

# problem constants (hardcoded per harness contract)
B, T, L = 32, 128, 16
VOCAB, E, H = 128, 512, 256
G4 = 4 * H  # 1024
N_CORES = 8
NW = (B * T) // N_CORES  # 512 words per core

F32 = mybir.dt.float32
DT = mybir.dt.bfloat16
F8 = mybir.dt.float8e4

AFT = mybir.ActivationFunctionType
ALU = mybir.AluOpType

# fp8 scaling: whh is stored *16, h is stored *64; LUT carries *1024 so the
# PSUM gate pre-activations are uniformly 1024*true. Powers of two => exact.
W_SCALE = 16.0
H_SCALE = 64.0
G_SCALE = W_SCALE * H_SCALE  # 1024
G_INV = 1.0 / G_SCALE

# deg-5 odd minimax-ish tanh coefficients, fit on |x| <= 0.6 (gate range is
# |x| <= 0.48): tanh(x) ~= x*(K0 + K1 x^2 + K2 x^4), |err| < 5e-5.
TANH_K = (0.99983975, -0.32921287, 0.10668909)


# --- custom DVE ops --------------------------------------------------------
def _register_custom_ops():
    from concourse.dve_spec import Spec, Src0, Src1, sq, lower, _has_src1
    from concourse.dve_spec import C0, C1, C2
    from concourse.dve_uop import DveOpSpec
    import concourse.dve_ops as dve_ops

    def reg(name, spec, subdim=False):
        for op in dve_ops.OPS:
            if op.name == name:
                return op
        row = max(dve_ops._SUB_OPCODE_FOR_NAME.values()) + 1
        assert row < 0x20
        shas = {}
        for ver in ("v3", "v4"):
            compiled = DveOpSpec(
                name=name, opcode=row, uops=lower(spec, ver=ver),
                rd1_en=_has_src1(spec),
            )
            shas[ver] = compiled.sha(ver)
        op = dve_ops.DveOp(name, spec, subdim=subdim, uops_sha=shas)
        dve_ops.OPS.append(op)
        dve_ops.CUSTOM_DVE_SPECS[name] = spec
        dve_ops._SUB_OPCODE_FOR_NAME[name] = row
        return op

    t0 = sq(Src0)
    poly = C0 + t0 * (C1 + t0 * C2)

    def _ref_tanh_ps(in0, in1, c0, c1, c2):
        x = in0.astype(np.float32)
        t = x * x
        return x * (c0 + t * (c1 + t * c2))

    tanh_ps = reg(
        "TANH_PS_ANT",
        Spec(body=Src0 * poly, reference=_ref_tanh_ps),
    )

    def _ref_tanh_mul(in0, in1, c0, c1, c2):
        x = in0.astype(np.float32)
        t = x * x
        return (x * (c0 + t * (c1 + t * c2))) * in1.astype(np.float32)

    tanh_mul = reg(
        "TANH_MUL_ANT",
        Spec(body=(Src0 * poly) * Src1, reference=_ref_tanh_mul),
    )
    return tanh_ps, tanh_mul


TANH_PS_OP, TANH_MUL_OP = _register_custom_ops()


def build_nc():
    nc = bacc.Bacc()

    # onehots: t=0 and t=15 land first (tiny DMAs) so both directions can
    # start immediately; the middle steps come as two bulk DMAs.
    oh_d = nc.dram_tensor("oh", [L, VOCAB, NW], DT, kind="ExternalInput")
    fused_d = nc.dram_tensor("fused", [2, VOCAB, G4], DT, kind="ExternalInput")
    whh_d = nc.dram_tensor("whh", [2, 2, 128, G4], F8, kind="ExternalInput")
    hout_d = nc.dram_tensor("hout", [128, 4 * NW], DT, kind="ExternalOutput")

    # tanh coefficients with the 2^-10 PSUM scale folded in (for PSUM-side
    # tanh of g: tanh(s*x) = x*(K0*s + K1*s^3 t + K2*s^5 t^2), t = x^2)
    s = G_INV
    g_c = (TANH_K[0] * s, TANH_K[1] * s**3, TANH_K[2] * s**5)
    # recurrent h = 64*tanh(c)*sig_o ; final h = tanh(c)*sig_o
    h_c = (TANH_K[0] * H_SCALE, TANH_K[1] * H_SCALE, TANH_K[2] * H_SCALE)
    f_c = TANH_K

    with tile.TileContext(nc) as tc:
        with (
            tc.tile_pool(name="const", bufs=1) as cpool,
            tc.tile_pool(name="work", bufs=2) as wpool,
            tc.tile_pool(name="state", bufs=2) as spool,
            tc.tile_pool(name="psum", bufs=2, space=bass.MemorySpace.PSUM) as ppool,
        ):
            # --- load constants (spread across engine DGE queues so the
            # descriptor setup doesn't serialize on one sequencer) ----------
            fused = {}
            whh = {}
            oh_ends = {}
            fu2 = cpool.tile([128, 2 * G4], DT, name="fused_sb", tag="fused")
            nc.sync.dma_start(
                fu2[:].rearrange("p (d g) -> p d g", d=2),
                fused_d.rearrange("d p g -> p d g"),
            )
            fused_off = {"f": 0, "b": G4}
            for eng, (d, te) in ((nc.sync, ("f", 0)), (nc.gpsimd, ("b", L - 1))):
                ot = cpool.tile([128, NW], DT, name=f"oh_e{te}", tag=f"oh_e{te}")
                eng.dma_start(ot[:], oh_d[te])
                oh_ends[te] = ot
            w2 = cpool.tile([128, 4 * G4], F8, name="whh_sb", tag="whh")
            nc.gpsimd.dma_start(
                w2[:].rearrange("p (d k g) -> p d k g", d=2, k=2),
                whh_d.rearrange("d k p g -> p d k g"),
            )
            whh_off = {"f": 0, "b": 2 * G4}
            oh_mid = {}
            for eng, (lo, hi) in ((nc.gpsimd, (1, 8)), (nc.gpsimd, (8, 15))):
                om = cpool.tile([128, 7 * NW], DT, name=f"oh_m{lo}", tag=f"oh_m{lo}")
                eng.dma_start(
                    om[:].rearrange("p (t n) -> p t n", t=7),
                    oh_d[lo:hi].rearrange("t p n -> p t n"),
                )
                oh_mid[lo] = om

            def oh_rhs(t):
                if t in oh_ends:
                    return oh_ends[t][:]
                lo = 1 if t < 8 else 8
                return oh_mid[lo][:, (t - lo) * NW : (t - lo + 1) * NW]

            out_sb = cpool.tile([128, 4 * NW], DT, name="out_sb", tag="out_sb")

            # HAM warm-up: dummy matmuls issued while the input DMAs are in
            # flight so the PE clock gate reaches 2.4 GHz before the first
            # real matmul. The one-hot tile for t=0 is tiny and lands first;
            # matmul on it (results overwritten by the start=True groups).
            warm_ps = ppool.tile([128, 4 * NW], F32, name="warm_ps", tag="ps")
            warm_src = oh_ends[0]
            for wj in range(30):
                nc.tensor.matmul(
                    warm_ps[:, (wj % 4) * NW : (wj % 4) * NW + 128],
                    warm_src[:, 0:128],
                    warm_src[:, 0:128],
                    start=True,
                    stop=True,
                )

            c_cur = {"f": None, "b": None}
            h_cur = {"f": None, "b": None}

            # gate chunk layout (torch (i,f,g,o) -> device (i,f,o,g) via PERM):
            # global chunks 0..7 = i0,i1,f0,f1,o0,o1,g0,g1
            # psum_a slices: i0,i1,f0,f1 (gc 0..3)
            # psum_b slices: g0,g1,o0,o1 (gc 6,7,4,5) -- g first so tanh(g)
            #   can start while the o matmuls still run
            B_GC = (6, 7, 4, 5)

            def emit_mms(d, t):
                tchar = t if d == "f" else L - 1 - t
                rhs_oh = oh_rhs(tchar)
                h_prev = h_cur[d]
                psum_a = ppool.tile([128, 4 * NW], F32, name="psum_a", tag="ps")
                psum_b = ppool.tile([128, 4 * NW], F32, name="psum_b", tag="ps")
                if h_prev is not None:
                    rhs_h = h_prev[:].rearrange("p (k n) -> p k n", k=2)
                    w3 = w2[:, whh_off[d] : whh_off[d] + 2 * G4].rearrange(
                        "p (k g) -> p k g", k=2
                    )
                # All one-hot (LUT) matmuls first: they depend only on
                # constants, so the PE can run them while the previous step's
                # cell chain still computes h; the DoubleRow h matmuls follow.
                # psum_a (i,f) before psum_b (g,o): sigmoid(if) heads the
                # cell critical path.
                fo = fused_off[d]
                for ps, gcs in ((psum_a, (0, 1, 2, 3)), (psum_b, B_GC)):
                    for jj, gc in enumerate(gcs):
                        sl = ps[:, jj * NW : (jj + 1) * NW]
                        lhs_f = fu2[:, fo + gc * 128 : fo + (gc + 1) * 128]
                        nc.tensor.matmul(
                            sl, lhs_f, rhs_oh, start=True, stop=h_prev is None
                        )
                if h_prev is not None:
                    for ps, gcs in ((psum_a, (0, 1, 2, 3)), (psum_b, B_GC)):
                        for jj, gc in enumerate(gcs):
                            sl = ps[:, jj * NW : (jj + 1) * NW]
                            lhs_h = w3[:, :, gc * 128 : (gc + 1) * 128]
                            nc.tensor.matmul(
                                sl,
                                lhs_h,
                                rhs_h,
                                start=False,
                                stop=True,
                                perf_mode=mybir.MatmulPerfMode.DoubleRow,
                            )
                return psum_a, psum_b

            def emit_sig_if(d, psum_a):
                sig_if = wpool.tile([128, 4 * NW], DT, name="sig_if", tag=f"sig_if_{d}")
                nc.scalar.activation(sig_if[:], psum_a[:], AFT.Sigmoid, scale=G_INV)
                return sig_if

            def emit_sig_o(d, psum_b):
                sig_o = wpool.tile([128, 2 * NW], DT, name="sig_o", tag=f"sig_o_{d}")
                nc.scalar.activation(
                    sig_o[:], psum_b[:, 2 * NW : 4 * NW], AFT.Sigmoid, scale=G_INV
                )
                return sig_o

            def emit_tanh_g_act(d, psum_b):
                tg = wpool.tile([128, 2 * NW], DT, name="tg", tag=f"tg_{d}")
                nc.scalar.activation(
                    tg[:], psum_b[:, 0 : 2 * NW], AFT.Tanh, scale=G_INV
                )
                return tg

            def emit_cell(d, sig_if, tg=None, psum_b=None, m1_pool=False):
                # c = sig(f)*c_prev + sig(i)*tanh(g). When tg is None, the
                # tanh(g)*sig(i) product comes fused from one custom DVE op
                # reading g straight from PSUM. m1_pool routes the sig(f)*c
                # product to GpSimd so it overlaps the DVE chain.
                c_prev = c_cur[d]
                c_new = spool.tile([128, 2 * NW], DT, name=f"c_{d}", tag=f"c_{d}")
                m2_dst = c_new if c_prev is None else wpool.tile(
                    [128, 2 * NW], DT, name="m2", tag=f"m2_{d}"
                )
                if c_prev is not None:
                    m1 = wpool.tile([128, 2 * NW], DT, name="m1", tag=f"m1_{d}")
                    eng = nc.gpsimd if m1_pool else nc.vector
                    eng.tensor_mul(m1[:], sig_if[:, 2 * NW : 4 * NW], c_prev[:])
                if tg is not None:
                    nc.vector.tensor_mul(m2_dst[:], tg[:], sig_if[:, 0 : 2 * NW])
                else:
                    nc.vector._custom_dve(
                        TANH_MUL_OP, out=m2_dst[:], in0=psum_b[:, 0 : 2 * NW],
                        in1=sig_if[:, 0 : 2 * NW],
                        s0=g_c[0], s1=g_c[1], imm2=g_c[2],
                    )
                if c_prev is not None:
                    # c_new = m1 + m2 on the otherwise-idle GpSimd engine;
                    # its consumers (tanh/h and next step's m1) have slack.
                    nc.gpsimd.tensor_add(c_new[:], m1[:], m2_dst[:])
                c_cur[d] = c_new

            def emit_h(d, t, sig_o, c_tile):
                # h = sig(o) * tanh(c); recurrent h is fp8 * 64, final is bf16
                if t == L - 1:
                    off = 0 if d == "f" else 2 * NW
                    nc.vector._custom_dve(
                        TANH_MUL_OP,
                        out=out_sb[:, off : off + 2 * NW],
                        in0=c_tile[:], in1=sig_o[:],
                        s0=f_c[0], s1=f_c[1], imm2=f_c[2],
                    )
                else:
                    h_new = spool.tile([128, 2 * NW], F8, name=f"h_{d}", tag=f"h_{d}")
                    nc.vector._custom_dve(
                        TANH_MUL_OP, out=h_new[:], in0=c_tile[:], in1=sig_o[:],
                        s0=h_c[0], s1=h_c[1], imm2=h_c[2],
                    )
                    h_cur[d] = h_new

            pending_b = None  # (t, sig_o_b, c_tile) awaiting next iteration
            for t in range(L):
                pa_f, pb_f = emit_mms("f", t)
                if pending_b is not None:
                    pt, p_sig_o, p_c = pending_b
                    emit_h("b", pt, p_sig_o, p_c)  # DVE: unblocks b's DR mms
                sig_if_f = emit_sig_if("f", pa_f)  # ACT
                sig_o_f = emit_sig_o("f", pb_f)    # ACT
                pa_b, pb_b = emit_mms("b", t)
                emit_cell("f", sig_if_f, psum_b=pb_f)  # DVE + GpSimd add
                emit_h("f", t, sig_o_f, c_cur["f"])  # DVE
                if t == L - 1:
                    nc.sync.dma_start(hout_d[:, 0 : 2 * NW], out_sb[:, 0 : 2 * NW])
                sig_if_b = emit_sig_if("b", pa_b)  # ACT
                sig_o_b = emit_sig_o("b", pb_b)    # ACT
                emit_cell("b", sig_if_b, psum_b=pb_b)
                pending_b = (t, sig_o_b, c_cur["b"])
            pt, p_sig_o, p_c = pending_b
            emit_h("b", pt, p_sig_o, p_c)

            nc.scalar.dma_start(hout_d[:, 2 * NW : 4 * NW], out_sb[:, 2 * NW : 4 * NW])

    nc.compile()
    return nc


_NC_CACHE = None


def _get_nc():
    global _NC_CACHE
    if _NC_CACHE is None:
        _NC_CACHE = build_nc()
    return _NC_CACHE


# gate permutation: torch order (i,f,g,o) -> device order (i,f,o,g)
_PERM = np.concatenate([np.arange(0, 512), np.arange(768, 1024), np.arange(512, 768)])


def _np_dt(dt):
    return mybir.dt.np(dt)


def prepare_in_maps(x, embed_table, w_ih_f, w_hh_f, b_ih_f, b_hh_f,
                    w_ih_b, w_hh_b, b_ih_b, b_hh_b):
    cdt = _np_dt(DT)
    f8dt = _np_dt(F8)
    ids = np.asarray(x).reshape(B * T, L).astype(np.int64)

    shared = {}
    fused_all = np.empty((2, VOCAB, G4), cdt)
    whh_all = np.empty((2, 2, 128, G4), f8dt)
    for di, (w_ih, w_hh, b_ih, b_hh) in enumerate(
        ((w_ih_f, w_hh_f, b_ih_f, b_hh_f), (w_ih_b, w_hh_b, b_ih_b, b_hh_b))
    ):
        w_ih = np.asarray(w_ih, np.float32)[_PERM]
        w_hh = np.asarray(w_hh, np.float32)[_PERM]
        b = (np.asarray(b_ih, np.float32) + np.asarray(b_hh, np.float32))[_PERM]
        fused = (np.asarray(embed_table, np.float32) @ w_ih.T + b[None, :]) * G_SCALE
        fused_all[di] = fused.astype(cdt)
        whh_all[di] = (w_hh.T * W_SCALE).reshape(2, 128, G4).astype(f8dt)
    shared["fused"] = fused_all
    shared["whh"] = whh_all

    vrange = np.arange(VOCAB)
    in_maps = []
    for c in range(N_CORES):
        ids_c = ids[c * NW : (c + 1) * NW]  # [NW, L]
        oh = (ids_c.T[:, None, :] == vrange[None, :, None]).astype(cdt)  # [L,V,NW]
        m = dict(shared)
        m["oh"] = np.ascontiguousarray(oh)
        in_maps.append(m)
    return in_maps


def assemble_output(results):
    ys = []
    for c in range(N_CORES):
        hout = results[c]["hout"].astype(np.float32)  # [128, 4*NW]
        hf = np.concatenate([hout[:, 0:NW], hout[:, NW : 2 * NW]], axis=0)  # [H,NW]
        hb = np.concatenate([hout[:, 2 * NW : 3 * NW], hout[:, 3 * NW : 4 * NW]], axis=0)
        ys.append(np.concatenate([hf.T, hb.T], axis=1))  # [NW, 2H]
    y = np.concatenate(ys, axis=0)  # [B*T, 2H]
    return y.reshape(B, T, 2 * H)


def run(in_maps, trace=False):
    nc = _get_nc()
    res = run_bass_kernel_spmd(nc, in_maps, core_ids=list(range(N_CORES)), trace=trace)
    return res


def kernel(**inputs) -> np.ndarray:
    in_maps = prepare_in_maps(**inputs)
    res = run(in_maps, trace=False)
    return assemble_output(res.results)


# revision 27
# speedup vs baseline: 1.1305x; 1.1305x over previous
"""Char-level BiLSTM embedder on 8 NeuronCores (Trainium2, Bass/Tile).

Computation: x[B=32,T=128,L=16] char ids -> embed[E=512] -> fwd+bwd LSTM(H=256)
over the L=16 chars of each of the N=B*T=4096 independent words -> final hidden
states concatenated -> y[B,T,2H=512].

Strategy (v2 — balanced 4-engine pipeline):
  - Data parallel over N: 512 words per core.
  - Embedding lookup + input projection + bias fused on HOST into a single
    [V=128, 4H] LUT per direction; on device the per-step input contribution
    is a K=128 bf16 matmul with a one-hot rhs, accumulated in PSUM.
  - Recurrent matmuls run in fp8e4m3 with perf_mode=DoubleRow: one K=256
    matmul per gate chunk (~1.44x over bf16 at FD=512). h is stored fp8
    scaled by 64 and w_hh scaled by 16; the LUT carries the matching 2^10
    scale and activations divide it back out (exact powers of two).
  - Activation work split across engines:
      ACT:  sigmoid(i,f) one batched op + sigmoid(o), reading PSUM with
            scale=2^-10.
      DVE:  custom fused ops (registered in dve_ops at import):
            TANH_PS_ANT  = deg-5 odd tanh poly straight from PSUM
            TANH_MUL_ANT = tanh(c)*sig_o fused (writes recurrent h in fp8)
            plus scalar_tensor_tensor (4x-capable) for the cell update.
    Gate pre-activations are tiny (|x| <= 0.5) so the deg-5 poly is exact
    to ~4e-5.
  - Gate order permuted to (i,f,o,g); psum_a=[i,f] psum_b=[g,o] so tanh(g)
    can drain early while o still accumulates.
  - fwd and bwd directions interleave per step to hide recurrence latency.
"""

import sys

sys.path.insert(0, "/opt/trn_rl_repo")

import numpy as np
import concourse.bass as bass
import concourse.bacc as bacc
import concourse.mybir as mybir
import concourse.tile as tile
from concourse.bass_utils import run_bass_kernel_spmd

# problem constants (hardcoded per harness contract)
B, T, L = 32, 128, 16
VOCAB, E, H = 128, 512, 256
G4 = 4 * H  # 1024
N_CORES = 8
NW = (B * T) // N_CORES  # 512 words per core

F32 = mybir.dt.float32
DT = mybir.dt.bfloat16
F8 = mybir.dt.float8e4

AFT = mybir.ActivationFunctionType
ALU = mybir.AluOpType

# fp8 scaling: whh is stored *16, h is stored *64; LUT carries *1024 so the
# PSUM gate pre-activations are uniformly 1024*true. Powers of two => exact.
W_SCALE = 16.0
H_SCALE = 64.0
G_SCALE = W_SCALE * H_SCALE  # 1024
G_INV = 1.0 / G_SCALE

# deg-5 odd minimax-ish tanh coefficients, fit on |x| <= 0.6 (gate range is
# |x| <= 0.48): tanh(x) ~= x*(K0 + K1 x^2 + K2 x^4), |err| < 5e-5.
TANH_K = (0.99983975, -0.32921287, 0.10668909)


# --- custom DVE ops --------------------------------------------------------
def _register_custom_ops():
    from concourse.dve_spec import Spec, Src0, Src1, sq, lower, _has_src1
    from concourse.dve_spec import C0, C1, C2
    from concourse.dve_uop import DveOpSpec
    import concourse.dve_ops as dve_ops

    def reg(name, spec, subdim=False):
        for op in dve_ops.OPS:
            if op.name == name:
                return op
        row = max(dve_ops._SUB_OPCODE_FOR_NAME.values()) + 1
        assert row < 0x20
        shas = {}
        for ver in ("v3", "v4"):
            compiled = DveOpSpec(
                name=name, opcode=row, uops=lower(spec, ver=ver),
                rd1_en=_has_src1(spec),
            )
            shas[ver] = compiled.sha(ver)
        op = dve_ops.DveOp(name, spec, subdim=subdim, uops_sha=shas)
        dve_ops.OPS.append(op)
        dve_ops.CUSTOM_DVE_SPECS[name] = spec
        dve_ops._SUB_OPCODE_FOR_NAME[name] = row
        return op

    t0 = sq(Src0)
    poly = C0 + t0 * (C1 + t0 * C2)

    def _ref_tanh_ps(in0, in1, c0, c1, c2):
        x = in0.astype(np.float32)
        t = x * x
        return x * (c0 + t * (c1 + t * c2))

    tanh_ps = reg(
        "TANH_PS_ANT",
        Spec(body=Src0 * poly, reference=_ref_tanh_ps),
    )

    def _ref_tanh_mul(in0, in1, c0, c1, c2):
        x = in0.astype(np.float32)
        t = x * x
        return (x * (c0 + t * (c1 + t * c2))) * in1.astype(np.float32)

    tanh_mul = reg(
        "TANH_MUL_ANT",
        Spec(body=(Src0 * poly) * Src1, reference=_ref_tanh_mul),
    )
    return tanh_ps, tanh_mul


TANH_PS_OP, TANH_MUL_OP = _register_custom_ops()


def build_nc():
    nc = bacc.Bacc()

    # onehots: t=0 and t=15 land first (tiny DMAs) so both directions can
    # start immediately; the middle steps come as two bulk DMAs.
    oh_d = nc.dram_tensor("oh", [L, VOCAB, NW], DT, kind="ExternalInput")
    fused_d = nc.dram_tensor("fused", [2, VOCAB, G4], DT, kind="ExternalInput")
    whh_d = nc.dram_tensor("whh", [2, 2, 128, G4], F8, kind="ExternalInput")
    hout_d = nc.dram_tensor("hout", [128, 4 * NW], DT, kind="ExternalOutput")

    # tanh coefficients with the 2^-10 PSUM scale folded in (for PSUM-side
    # tanh of g: tanh(s*x) = x*(K0*s + K1*s^3 t + K2*s^5 t^2), t = x^2)
    s = G_INV
    g_c = (TANH_K[0] * s, TANH_K[1] * s**3, TANH_K[2] * s**5)
    # recurrent h = 64*tanh(c)*sig_o ; final h = tanh(c)*sig_o
    h_c = (TANH_K[0] * H_SCALE, TANH_K[1] * H_SCALE, TANH_K[2] * H_SCALE)
    f_c = TANH_K

    with tile.TileContext(nc) as tc:
        with (
            tc.tile_pool(name="const", bufs=1) as cpool,
            tc.tile_pool(name="work", bufs=2) as wpool,
            tc.tile_pool(name="state", bufs=2) as spool,
            tc.tile_pool(name="psum", bufs=2, space=bass.MemorySpace.PSUM) as ppool,
        ):
            # --- load constants (spread across engine DGE queues so the
            # descriptor setup doesn't serialize on one sequencer) ----------
            fused = {}
            whh = {}
            oh_ends = {}
            fu2 = cpool.tile([128, 2 * G4], DT, name="fused_sb", tag="fused")
            nc.sync.dma_start(
                fu2[:].rearrange("p (d g) -> p d g", d=2),
                fused_d.rearrange("d p g -> p d g"),
            )
            fused_off = {"f": 0, "b": G4}
            for eng, (d, te) in ((nc.sync, ("f", 0)), (nc.gpsimd, ("b", L - 1))):
                ot = cpool.tile([128, NW], DT, name=f"oh_e{te}", tag=f"oh_e{te}")
                eng.dma_start(ot[:], oh_d[te])
                oh_ends[te] = ot
            w2 = cpool.tile([128, 4 * G4], F8, name="whh_sb", tag="whh")
            nc.gpsimd.dma_start(
                w2[:].rearrange("p (d k g) -> p d k g", d=2, k=2),
                whh_d.rearrange("d k p g -> p d k g"),
            )
            whh_off = {"f": 0, "b": 2 * G4}
            oh_mid = {}
            for eng, (lo, hi) in ((nc.gpsimd, (1, 8)), (nc.gpsimd, (8, 15))):
                om = cpool.tile([128, 7 * NW], DT, name=f"oh_m{lo}", tag=f"oh_m{lo}")
                eng.dma_start(
                    om[:].rearrange("p (t n) -> p t n", t=7),
                    oh_d[lo:hi].rearrange("t p n -> p t n"),
                )
                oh_mid[lo] = om

            def oh_rhs(t):
                if t in oh_ends:
                    return oh_ends[t][:]
                lo = 1 if t < 8 else 8
                return oh_mid[lo][:, (t - lo) * NW : (t - lo + 1) * NW]

            out_sb = cpool.tile([128, 4 * NW], DT, name="out_sb", tag="out_sb")

            # HAM warm-up: dummy matmuls issued while the input DMAs are in
            # flight so the PE clock gate reaches 2.4 GHz before the first
            # real matmul. The one-hot tile for t=0 is tiny and lands first;
            # matmul on it (results overwritten by the start=True groups).
            warm_ps = ppool.tile([128, 4 * NW], F32, name="warm_ps", tag="ps")
            warm_src = oh_ends[0]
            for wj in range(30):
                nc.tensor.matmul(
                    warm_ps[:, (wj % 4) * NW : (wj % 4) * NW + 128],
                    warm_src[:, 0:128],
                    warm_src[:, 0:128],
                    start=True,
                    stop=True,
                )

            c_cur = {"f": None, "b": None}
            h_cur = {"f": None, "b": None}

            # gate chunk layout (torch (i,f,g,o) -> device (i,f,o,g) via PERM):
            # global chunks 0..7 = i0,i1,f0,f1,o0,o1,g0,g1
            # psum_a slices: i0,i1,f0,f1 (gc 0..3)
            # psum_b slices: g0,g1,o0,o1 (gc 6,7,4,5) -- g first so tanh(g)
            #   can start while the o matmuls still run
            B_GC = (6, 7, 4, 5)

            def emit_mms(d, t):
                tchar = t if d == "f" else L - 1 - t
                rhs_oh = oh_rhs(tchar)
                h_prev = h_cur[d]
                psum_a = ppool.tile([128, 4 * NW], F32, name="psum_a", tag="ps")
                psum_b = ppool.tile([128, 4 * NW], F32, name="psum_b", tag="ps")
                if h_prev is not None:
                    rhs_h = h_prev[:].rearrange("p (k n) -> p k n", k=2)
                    w3 = w2[:, whh_off[d] : whh_off[d] + 2 * G4].rearrange(
                        "p (k g) -> p k g", k=2
                    )
                # All one-hot (LUT) matmuls first: they depend only on
                # constants, so the PE can run them while the previous step's
                # cell chain still computes h; the DoubleRow h matmuls follow.
                # psum_a (i,f) before psum_b (g,o): sigmoid(if) heads the
                # cell critical path.
                fo = fused_off[d]
                for ps, gcs in ((psum_a, (0, 1, 2, 3)), (psum_b, B_GC)):
                    for jj, gc in enumerate(gcs):
                        sl = ps[:, jj * NW : (jj + 1) * NW]
                        lhs_f = fu2[:, fo + gc * 128 : fo + (gc + 1) * 128]
                        nc.tensor.matmul(
                            sl, lhs_f, rhs_oh, start=True, stop=h_prev is None
                        )
                if h_prev is not None:
                    for ps, gcs in ((psum_a, (0, 1, 2, 3)), (psum_b, B_GC)):
                        for jj, gc in enumerate(gcs):
                            sl = ps[:, jj * NW : (jj + 1) * NW]
                            lhs_h = w3[:, :, gc * 128 : (gc + 1) * 128]
                            nc.tensor.matmul(
                                sl,
                                lhs_h,
                                rhs_h,
                                start=False,
                                stop=True,
                                perf_mode=mybir.MatmulPerfMode.DoubleRow,
                            )
                return psum_a, psum_b

            def emit_sig_if(d, psum_a):
                sig_if = wpool.tile([128, 4 * NW], DT, name="sig_if", tag=f"sig_if_{d}")
                nc.scalar.activation(sig_if[:], psum_a[:], AFT.Sigmoid, scale=G_INV)
                return sig_if

            def emit_sig_o(d, psum_b):
                sig_o = wpool.tile([128, 2 * NW], DT, name="sig_o", tag=f"sig_o_{d}")
                nc.scalar.activation(
                    sig_o[:], psum_b[:, 2 * NW : 4 * NW], AFT.Sigmoid, scale=G_INV
                )
                return sig_o

            def emit_tanh_g_act(d, psum_b):
                tg = wpool.tile([128, 2 * NW], DT, name="tg", tag=f"tg_{d}")
                nc.scalar.activation(
                    tg[:], psum_b[:, 0 : 2 * NW], AFT.Tanh, scale=G_INV
                )
                return tg

            def emit_cell(d, sig_if, tg=None, psum_b=None, m1_pool=False):
                # c = sig(f)*c_prev + sig(i)*tanh(g). When tg is None, the
                # tanh(g)*sig(i) product comes fused from one custom DVE op
                # reading g straight from PSUM. m1_pool routes the sig(f)*c
                # product to GpSimd so it overlaps the DVE chain.
                c_prev = c_cur[d]
                c_new = spool.tile([128, 2 * NW], DT, name=f"c_{d}", tag=f"c_{d}")
                m2_dst = c_new if c_prev is None else wpool.tile(
                    [128, 2 * NW], DT, name="m2", tag=f"m2_{d}"
                )
                if c_prev is not None:
                    m1 = wpool.tile([128, 2 * NW], DT, name="m1", tag=f"m1_{d}")
                    eng = nc.gpsimd if m1_pool else nc.vector
                    eng.tensor_mul(m1[:], sig_if[:, 2 * NW : 4 * NW], c_prev[:])
                if tg is not None:
                    nc.vector.tensor_mul(m2_dst[:], tg[:], sig_if[:, 0 : 2 * NW])
                else:
                    nc.vector._custom_dve(
                        TANH_MUL_OP, out=m2_dst[:], in0=psum_b[:, 0 : 2 * NW],
                        in1=sig_if[:, 0 : 2 * NW],
                        s0=g_c[0], s1=g_c[1], imm2=g_c[2],
                    )
                if c_prev is not None:
                    nc.vector.tensor_add(c_new[:], m1[:], m2_dst[:])
                c_cur[d] = c_new

            def emit_h(d, t, sig_o, c_tile):
                # h = sig(o) * tanh(c); recurrent h is fp8 * 64, final is bf16
                if t == L - 1:
                    off = 0 if d == "f" else 2 * NW
                    nc.vector._custom_dve(
                        TANH_MUL_OP,
                        out=out_sb[:, off : off + 2 * NW],
                        in0=c_tile[:], in1=sig_o[:],
                        s0=f_c[0], s1=f_c[1], imm2=f_c[2],
                    )
                else:
                    h_new = spool.tile([128, 2 * NW], F8, name=f"h_{d}", tag=f"h_{d}")
                    nc.vector._custom_dve(
                        TANH_MUL_OP, out=h_new[:], in0=c_tile[:], in1=sig_o[:],
                        s0=h_c[0], s1=h_c[1], imm2=h_c[2],
                    )
                    h_cur[d] = h_new

            pending_b = None  # (t, sig_o_b, c_tile) awaiting next iteration
            for t in range(L):
                pa_f, pb_f = emit_mms("f", t)
                if pending_b is not None:
                    pt, p_sig_o, p_c = pending_b
                    emit_h("b", pt, p_sig_o, p_c)  # DVE: unblocks b's DR mms
                sig_if_f = emit_sig_if("f", pa_f)  # ACT
                sig_o_f = emit_sig_o("f", pb_f)    # ACT (early: frees psum_b)
                tg_f = emit_tanh_g_act("f", pb_f)  # ACT
                pa_b, pb_b = emit_mms("b", t)
                emit_cell("f", sig_if_f, tg=tg_f)  # DVE x3
                emit_h("f", t, sig_o_f, c_cur["f"])  # DVE
                if t == L - 1:
                    nc.sync.dma_start(hout_d[:, 0 : 2 * NW], out_sb[:, 0 : 2 * NW])
                sig_if_b = emit_sig_if("b", pa_b)  # ACT
                sig_o_b = emit_sig_o("b", pb_b)    # ACT
                emit_cell("b", sig_if_b, psum_b=pb_b)
                pending_b = (t, sig_o_b, c_cur["b"])
            pt, p_sig_o, p_c = pending_b
            emit_h("b", pt, p_sig_o, p_c)

            nc.scalar.dma_start(hout_d[:, 2 * NW : 4 * NW], out_sb[:, 2 * NW : 4 * NW])

    nc.compile()
    return nc


_NC_CACHE = None


def _get_nc():
    global _NC_CACHE
    if _NC_CACHE is None:
        _NC_CACHE = build_nc()
    return _NC_CACHE


# gate permutation: torch order (i,f,g,o) -> device order (i,f,o,g)
_PERM = np.concatenate([np.arange(0, 512), np.arange(768, 1024), np.arange(512, 768)])


def _np_dt(dt):
    return mybir.dt.np(dt)


def prepare_in_maps(x, embed_table, w_ih_f, w_hh_f, b_ih_f, b_hh_f,
                    w_ih_b, w_hh_b, b_ih_b, b_hh_b):
    cdt = _np_dt(DT)
    f8dt = _np_dt(F8)
    ids = np.asarray(x).reshape(B * T, L).astype(np.int64)

    shared = {}
    fused_all = np.empty((2, VOCAB, G4), cdt)
    whh_all = np.empty((2, 2, 128, G4), f8dt)
    for di, (w_ih, w_hh, b_ih, b_hh) in enumerate(
        ((w_ih_f, w_hh_f, b_ih_f, b_hh_f), (w_ih_b, w_hh_b, b_ih_b, b_hh_b))
    ):
        w_ih = np.asarray(w_ih, np.float32)[_PERM]
        w_hh = np.asarray(w_hh, np.float32)[_PERM]
        b = (np.asarray(b_ih, np.float32) + np.asarray(b_hh, np.float32))[_PERM]
        fused = (np.asarray(embed_table, np.float32) @ w_ih.T + b[None, :]) * G_SCALE
        fused_all[di] = fused.astype(cdt)
        whh_all[di] = (w_hh.T * W_SCALE).reshape(2, 128, G4).astype(f8dt)
    shared["fused"] = fused_all
    shared["whh"] = whh_all

    vrange = np.arange(VOCAB)
    in_maps = []
    for c in range(N_CORES):
        ids_c = ids[c * NW : (c + 1) * NW]  # [NW, L]
        oh = (ids_c.T[:, None, :] == vrange[None, :, None]).astype(cdt)  # [L,V,NW]
        m = dict(shared)
        m["oh"] = np.ascontiguousarray(oh)
        in_maps.append(m)
    return in_maps


def assemble_output(results):
    ys = []
    for c in range(N_CORES):
        hout = results[c]["hout"].astype(np.float32)  # [128, 4*NW]
        hf = np.concatenate([hout[:, 0:NW], hout[:, NW : 2 * NW]], axis=0)  # [H,NW]
        hb = np.concatenate([hout[:, 2 * NW : 3 * NW], hout[:, 3 * NW : 4 * NW]], axis=0)
        ys.append(np.concatenate([hf.T, hb.T], axis=1))  # [NW, 2H]
    y = np.concatenate(ys, axis=0)  # [B*T, 2H]
    return y.reshape(B, T, 2 * H)


def run(in_maps, trace=False):
    nc = _get_nc()
    res = run_bass_kernel_spmd(nc, in_maps, core_ids=list(range(N_CORES)), trace=trace)
    return res


def kernel(**inputs) -> np.ndarray:
    in_maps = prepare_in_maps(**inputs)
    res = run(in_maps, trace=False)
    return assemble_output(res.results)


# revision 29
# speedup vs baseline: 1.1657x; 1.0311x over previous
"""Char-level BiLSTM embedder on 8 NeuronCores (Trainium2, Bass/Tile).

Computation: x[B=32,T=128,L=16] char ids -> embed[E=512] -> fwd+bwd LSTM(H=256)
over the L=16 chars of each of the N=B*T=4096 independent words -> final hidden
states concatenated -> y[B,T,2H=512].

Strategy (v2 — balanced 4-engine pipeline):
  - Data parallel over N: 512 words per core.
  - Embedding lookup + input projection + bias fused on HOST into a single
    [V=128, 4H] LUT per direction; on device the per-step input contribution
    is a K=128 bf16 matmul with a one-hot rhs, accumulated in PSUM.
  - Recurrent matmuls run in fp8e4m3 with perf_mode=DoubleRow: one K=256
    matmul per gate chunk (~1.44x over bf16 at FD=512). h is stored fp8
    scaled by 64 and w_hh scaled by 16; the LUT carries the matching 2^10
    scale and activations divide it back out (exact powers of two).
  - Activation work split across engines:
      ACT:  sigmoid(i,f) one batched op + sigmoid(o), reading PSUM with
            scale=2^-10.
      DVE:  custom fused ops (registered in dve_ops at import):
            TANH_PS_ANT  = deg-5 odd tanh poly straight from PSUM
            TANH_MUL_ANT = tanh(c)*sig_o fused (writes recurrent h in fp8)
            plus scalar_tensor_tensor (4x-capable) for the cell update.
    Gate pre-activations are tiny (|x| <= 0.5) so the deg-5 poly is exact
    to ~4e-5.
  - Gate order permuted to (i,f,o,g); psum_a=[i,f] psum_b=[g,o] so tanh(g)
    can drain early while o still accumulates.
  - fwd and bwd directions interleave per step to hide recurrence latency.
"""

import sys

sys.path.insert(0, "/opt/trn_rl_repo")

import numpy as np
import concourse.bass as bass
import concourse.bacc as bacc
import concourse.mybir as mybir
import concourse.tile as tile
from concourse.bass_utils import run_bass_kernel_spmd

# problem constants (hardcoded per harness contract)
B, T, L = 32, 128, 16
VOCAB, E, H = 128, 512, 256
G4 = 4 * H  # 1024
N_CORES = 8
NW = (B * T) // N_CORES  # 512 words per core

F32 = mybir.dt.float32
DT = mybir.dt.bfloat16
F8 = mybir.dt.float8e4

AFT = mybir.ActivationFunctionType
ALU = mybir.AluOpType

# fp8 scaling: whh is stored *16, h is stored *64; LUT carries *1024 so the
# PSUM gate pre-activations are uniformly 1024*true. Powers of two => exact.
W_SCALE = 16.0
H_SCALE = 64.0
G_SCALE = W_SCALE * H_SCALE  # 1024
G_INV = 1.0 / G_SCALE

# deg-5 odd minimax-ish tanh coefficients, fit on |x| <= 0.6 (gate range is
# |x| <= 0.48): tanh(x) ~= x*(K0 + K1 x^2 + K2 x^4), |err| < 5e-5.
TANH_K = (0.99983975, -0.32921287, 0.10668909)


# --- custom DVE ops --------------------------------------------------------
def _register_custom_ops():
    from concourse.dve_spec import Spec, Src0, Src1, sq, lower, _has_src1
    from concourse.dve_spec import C0, C1, C2
    from concourse.dve_uop import DveOpSpec
    import concourse.dve_ops as dve_ops

    def reg(name, spec, subdim=False):
        for op in dve_ops.OPS:
            if op.name == name:
                return op
        row = max(dve_ops._SUB_OPCODE_FOR_NAME.values()) + 1
        assert row < 0x20
        shas = {}
        for ver in ("v3", "v4"):
            compiled = DveOpSpec(
                name=name, opcode=row, uops=lower(spec, ver=ver),
                rd1_en=_has_src1(spec),
            )
            shas[ver] = compiled.sha(ver)
        op = dve_ops.DveOp(name, spec, subdim=subdim, uops_sha=shas)
        dve_ops.OPS.append(op)
        dve_ops.CUSTOM_DVE_SPECS[name] = spec
        dve_ops._SUB_OPCODE_FOR_NAME[name] = row
        return op

    t0 = sq(Src0)
    poly = C0 + t0 * (C1 + t0 * C2)

    def _ref_tanh_ps(in0, in1, c0, c1, c2):
        x = in0.astype(np.float32)
        t = x * x
        return x * (c0 + t * (c1 + t * c2))

    tanh_ps = reg(
        "TANH_PS_ANT",
        Spec(body=Src0 * poly, reference=_ref_tanh_ps),
    )

    def _ref_tanh_mul(in0, in1, c0, c1, c2):
        x = in0.astype(np.float32)
        t = x * x
        return (x * (c0 + t * (c1 + t * c2))) * in1.astype(np.float32)

    tanh_mul = reg(
        "TANH_MUL_ANT",
        Spec(body=(Src0 * poly) * Src1, reference=_ref_tanh_mul),
    )
    return tanh_ps, tanh_mul


TANH_PS_OP, TANH_MUL_OP = _register_custom_ops()


def build_nc():
    nc = bacc.Bacc()

    # onehots: t=0 and t=15 land first (tiny DMAs) so both directions can
    # start immediately; the middle steps come as two bulk DMAs.
    oh_d = nc.dram_tensor("oh", [L, VOCAB, NW], DT, kind="ExternalInput")
    fused_d = nc.dram_tensor("fused", [2, VOCAB, G4], DT, kind="ExternalInput")
    whh_d = nc.dram_tensor("whh", [2, 2, 128, G4], F8, kind="ExternalInput")
    hout_d = nc.dram_tensor("hout", [128, 4 * NW], DT, kind="ExternalOutput")

    # tanh coefficients with the 2^-10 PSUM scale folded in (for PSUM-side
    # tanh of g: tanh(s*x) = x*(K0*s + K1*s^3 t + K2*s^5 t^2), t = x^2)
    s = G_INV
    g_c = (TANH_K[0] * s, TANH_K[1] * s**3, TANH_K[2] * s**5)
    # recurrent h = 64*tanh(c)*sig_o ; final h = tanh(c)*sig_o
    h_c = (TANH_K[0] * H_SCALE, TANH_K[1] * H_SCALE, TANH_K[2] * H_SCALE)
    f_c = TANH_K

    with tile.TileContext(nc) as tc:
        with (
            tc.tile_pool(name="const", bufs=1) as cpool,
            tc.tile_pool(name="work", bufs=2) as wpool,
            tc.tile_pool(name="state", bufs=2) as spool,
            tc.tile_pool(name="psum", bufs=2, space=bass.MemorySpace.PSUM) as ppool,
        ):
            # --- load constants (spread across engine DGE queues so the
            # descriptor setup doesn't serialize on one sequencer) ----------
            fused = {}
            whh = {}
            oh_ends = {}
            for eng, (d, te) in ((nc.sync, ("f", 0)), (nc.gpsimd, ("b", L - 1))):
                ot = cpool.tile([128, NW], DT, name=f"oh_e{te}", tag=f"oh_e{te}")
                eng.dma_start(ot[:], oh_d[te])
                oh_ends[te] = ot
            fu2 = cpool.tile([128, 2 * G4], DT, name="fused_sb", tag="fused")
            nc.sync.dma_start(
                fu2[:].rearrange("p (d g) -> p d g", d=2),
                fused_d.rearrange("d p g -> p d g"),
            )
            fused_off = {"f": 0, "b": G4}
            w2 = cpool.tile([128, 4 * G4], F8, name="whh_sb", tag="whh")
            nc.gpsimd.dma_start(
                w2[:].rearrange("p (d k g) -> p d k g", d=2, k=2),
                whh_d.rearrange("d k p g -> p d k g"),
            )
            whh_off = {"f": 0, "b": 2 * G4}
            oh_mid = {}
            for eng, (lo, hi) in ((nc.gpsimd, (1, 8)), (nc.gpsimd, (8, 15))):
                om = cpool.tile([128, 7 * NW], DT, name=f"oh_m{lo}", tag=f"oh_m{lo}")
                eng.dma_start(
                    om[:].rearrange("p (t n) -> p t n", t=7),
                    oh_d[lo:hi].rearrange("t p n -> p t n"),
                )
                oh_mid[lo] = om

            def oh_rhs(t):
                if t in oh_ends:
                    return oh_ends[t][:]
                lo = 1 if t < 8 else 8
                return oh_mid[lo][:, (t - lo) * NW : (t - lo + 1) * NW]

            out_sb = cpool.tile([128, 4 * NW], DT, name="out_sb", tag="out_sb")

            # HAM warm-up: dummy matmuls issued while the input DMAs are in
            # flight so the PE clock gate reaches 2.4 GHz before the first
            # real matmul. The one-hot tile for t=0 is tiny and lands first;
            # matmul on it (results overwritten by the start=True groups).
            warm_ps = ppool.tile([128, 4 * NW], F32, name="warm_ps", tag="ps")
            warm_src = oh_ends[0]
            for wj in range(30):
                nc.tensor.matmul(
                    warm_ps[:, (wj % 4) * NW : (wj % 4) * NW + 128],
                    warm_src[:, 0:128],
                    warm_src[:, 0:128],
                    start=True,
                    stop=True,
                )

            c_cur = {"f": None, "b": None}
            h_cur = {"f": None, "b": None}

            # gate chunk layout (torch (i,f,g,o) -> device (i,f,o,g) via PERM):
            # global chunks 0..7 = i0,i1,f0,f1,o0,o1,g0,g1
            # psum_a slices: i0,i1,f0,f1 (gc 0..3)
            # psum_b slices: g0,g1,o0,o1 (gc 6,7,4,5) -- g first so tanh(g)
            #   can start while the o matmuls still run
            B_GC = (6, 7, 4, 5)

            def emit_mms(d, t):
                tchar = t if d == "f" else L - 1 - t
                rhs_oh = oh_rhs(tchar)
                h_prev = h_cur[d]
                psum_a = ppool.tile([128, 4 * NW], F32, name="psum_a", tag="ps")
                psum_b = ppool.tile([128, 4 * NW], F32, name="psum_b", tag="ps")
                if h_prev is not None:
                    rhs_h = h_prev[:].rearrange("p (k n) -> p k n", k=2)
                    w3 = w2[:, whh_off[d] : whh_off[d] + 2 * G4].rearrange(
                        "p (k g) -> p k g", k=2
                    )
                # All one-hot (LUT) matmuls first: they depend only on
                # constants, so the PE can run them while the previous step's
                # cell chain still computes h; the DoubleRow h matmuls follow.
                # psum_a (i,f) before psum_b (g,o): sigmoid(if) heads the
                # cell critical path.
                fo = fused_off[d]
                for ps, gcs in ((psum_a, (0, 1, 2, 3)), (psum_b, B_GC)):
                    for jj, gc in enumerate(gcs):
                        sl = ps[:, jj * NW : (jj + 1) * NW]
                        lhs_f = fu2[:, fo + gc * 128 : fo + (gc + 1) * 128]
                        nc.tensor.matmul(
                            sl, lhs_f, rhs_oh, start=True, stop=h_prev is None
                        )
                if h_prev is not None:
                    for ps, gcs in ((psum_a, (0, 1, 2, 3)), (psum_b, B_GC)):
                        for jj, gc in enumerate(gcs):
                            sl = ps[:, jj * NW : (jj + 1) * NW]
                            lhs_h = w3[:, :, gc * 128 : (gc + 1) * 128]
                            nc.tensor.matmul(
                                sl,
                                lhs_h,
                                rhs_h,
                                start=False,
                                stop=True,
                                perf_mode=mybir.MatmulPerfMode.DoubleRow,
                            )
                return psum_a, psum_b

            def emit_sig_if(d, psum_a):
                sig_if = wpool.tile([128, 4 * NW], DT, name="sig_if", tag=f"sig_if_{d}")
                nc.scalar.activation(sig_if[:], psum_a[:], AFT.Sigmoid, scale=G_INV)
                return sig_if

            def emit_sig_o(d, psum_b):
                sig_o = wpool.tile([128, 2 * NW], DT, name="sig_o", tag=f"sig_o_{d}")
                nc.scalar.activation(
                    sig_o[:], psum_b[:, 2 * NW : 4 * NW], AFT.Sigmoid, scale=G_INV
                )
                return sig_o

            def emit_tanh_g_act(d, psum_b):
                tg = wpool.tile([128, 2 * NW], DT, name="tg", tag=f"tg_{d}")
                nc.scalar.activation(
                    tg[:], psum_b[:, 0 : 2 * NW], AFT.Tanh, scale=G_INV
                )
                return tg

            def emit_cell(d, sig_if, tg=None, psum_b=None, m1_pool=False):
                # c = sig(f)*c_prev + sig(i)*tanh(g). When tg is None, the
                # tanh(g)*sig(i) product comes fused from one custom DVE op
                # reading g straight from PSUM. m1_pool routes the sig(f)*c
                # product to GpSimd so it overlaps the DVE chain.
                c_prev = c_cur[d]
                c_new = spool.tile([128, 2 * NW], DT, name=f"c_{d}", tag=f"c_{d}")
                m2_dst = c_new if c_prev is None else wpool.tile(
                    [128, 2 * NW], DT, name="m2", tag=f"m2_{d}"
                )
                if c_prev is not None:
                    m1 = wpool.tile([128, 2 * NW], DT, name="m1", tag=f"m1_{d}")
                    eng = nc.gpsimd if m1_pool else nc.vector
                    eng.tensor_mul(m1[:], sig_if[:, 2 * NW : 4 * NW], c_prev[:])
                if tg is not None:
                    nc.vector.tensor_mul(m2_dst[:], tg[:], sig_if[:, 0 : 2 * NW])
                else:
                    nc.vector._custom_dve(
                        TANH_MUL_OP, out=m2_dst[:], in0=psum_b[:, 0 : 2 * NW],
                        in1=sig_if[:, 0 : 2 * NW],
                        s0=g_c[0], s1=g_c[1], imm2=g_c[2],
                    )
                if c_prev is not None:
                    nc.vector.tensor_add(c_new[:], m1[:], m2_dst[:])
                c_cur[d] = c_new

            def emit_h(d, t, sig_o, c_tile):
                # h = sig(o) * tanh(c); recurrent h is fp8 * 64, final is bf16
                if t == L - 1:
                    off = 0 if d == "f" else 2 * NW
                    nc.vector._custom_dve(
                        TANH_MUL_OP,
                        out=out_sb[:, off : off + 2 * NW],
                        in0=c_tile[:], in1=sig_o[:],
                        s0=f_c[0], s1=f_c[1], imm2=f_c[2],
                    )
                else:
                    h_new = spool.tile([128, 2 * NW], F8, name=f"h_{d}", tag=f"h_{d}")
                    nc.vector._custom_dve(
                        TANH_MUL_OP, out=h_new[:], in0=c_tile[:], in1=sig_o[:],
                        s0=h_c[0], s1=h_c[1], imm2=h_c[2],
                    )
                    h_cur[d] = h_new

            pending_b = None  # (t, sig_o_b, c_tile) awaiting next iteration
            for t in range(L):
                pa_f, pb_f = emit_mms("f", t)
                if pending_b is not None:
                    pt, p_sig_o, p_c = pending_b
                    emit_h("b", pt, p_sig_o, p_c)  # DVE: unblocks b's DR mms
                sig_if_f = emit_sig_if("f", pa_f)  # ACT
                tg_f = emit_tanh_g_act("f", pb_f)  # ACT (balances engines)
                sig_o_f = emit_sig_o("f", pb_f)    # ACT
                pa_b, pb_b = emit_mms("b", t)
                emit_cell("f", sig_if_f, tg=tg_f)  # DVE x3
                emit_h("f", t, sig_o_f, c_cur["f"])  # DVE
                if t == L - 1:
                    nc.sync.dma_start(hout_d[:, 0 : 2 * NW], out_sb[:, 0 : 2 * NW])
                sig_if_b = emit_sig_if("b", pa_b)  # ACT
                sig_o_b = emit_sig_o("b", pb_b)    # ACT
                emit_cell("b", sig_if_b, psum_b=pb_b)
                pending_b = (t, sig_o_b, c_cur["b"])
            pt, p_sig_o, p_c = pending_b
            emit_h("b", pt, p_sig_o, p_c)

            nc.scalar.dma_start(hout_d[:, 2 * NW : 4 * NW], out_sb[:, 2 * NW : 4 * NW])

    nc.compile()
    return nc


_NC_CACHE = None


def _get_nc():
    global _NC_CACHE
    if _NC_CACHE is None:
        _NC_CACHE = build_nc()
    return _NC_CACHE


# gate permutation: torch order (i,f,g,o) -> device order (i,f,o,g)
_PERM = np.concatenate([np.arange(0, 512), np.arange(768, 1024), np.arange(512, 768)])


def _np_dt(dt):
    return mybir.dt.np(dt)


def prepare_in_maps(x, embed_table, w_ih_f, w_hh_f, b_ih_f, b_hh_f,
                    w_ih_b, w_hh_b, b_ih_b, b_hh_b):
    cdt = _np_dt(DT)
    f8dt = _np_dt(F8)
    ids = np.asarray(x).reshape(B * T, L).astype(np.int64)

    shared = {}
    fused_all = np.empty((2, VOCAB, G4), cdt)
    whh_all = np.empty((2, 2, 128, G4), f8dt)
    for di, (w_ih, w_hh, b_ih, b_hh) in enumerate(
        ((w_ih_f, w_hh_f, b_ih_f, b_hh_f), (w_ih_b, w_hh_b, b_ih_b, b_hh_b))
    ):
        w_ih = np.asarray(w_ih, np.float32)[_PERM]
        w_hh = np.asarray(w_hh, np.float32)[_PERM]
        b = (np.asarray(b_ih, np.float32) + np.asarray(b_hh, np.float32))[_PERM]
        fused = (np.asarray(embed_table, np.float32) @ w_ih.T + b[None, :]) * G_SCALE
        fused_all[di] = fused.astype(cdt)
        whh_all[di] = (w_hh.T * W_SCALE).reshape(2, 128, G4).astype(f8dt)
    shared["fused"] = fused_all
    shared["whh"] = whh_all

    vrange = np.arange(VOCAB)
    in_maps = []
    for c in range(N_CORES):
        ids_c = ids[c * NW : (c + 1) * NW]  # [NW, L]
        oh = (ids_c.T[:, None, :] == vrange[None, :, None]).astype(cdt)  # [L,V,NW]
        m = dict(shared)
        m["oh"] = np.ascontiguousarray(oh)
        in_maps.append(m)
    return in_maps


def assemble_output(results):
    ys = []
    for c in range(N_CORES):
        hout = results[c]["hout"].astype(np.float32)  # [128, 4*NW]
        hf = np.concatenate([hout[:, 0:NW], hout[:, NW : 2 * NW]], axis=0)  # [H,NW]
        hb = np.concatenate([hout[:, 2 * NW : 3 * NW], hout[:, 3 * NW : 4 * NW]], axis=0)
        ys.append(np.concatenate([hf.T, hb.T], axis=1))  # [NW, 2H]
    y = np.concatenate(ys, axis=0)  # [B*T, 2H]
    return y.reshape(B, T, 2 * H)


def run(in_maps, trace=False):
    nc = _get_nc()
    res = run_bass_kernel_spmd(nc, in_maps, core_ids=list(range(N_CORES)), trace=trace)
    return res


def kernel(**inputs) -> np.ndarray:
    in_maps = prepare_in_maps(**inputs)
    res = run(in_maps, trace=False)
    return assemble_output(res.results)
